# revision 1
# baseline (speedup 1.0000x reference)
"""MoE routing kernel (nn_DecFCSwitch) for 8 Trainium2 NeuronCores.

Reference computes all 16 expert branches for every token and then
selects one per token.  Only the selected branch matters, so:

  host:   sort tokens by expert, pad each expert's tokens to capacity C,
          relu(x) (the residual add also stays on host), transpose so
          the feature dim lands on SBUF partitions, cast to bf16.
  device: expert-parallel SPMD — core i owns experts {2i, 2i+1} and runs
          a 2-layer MLP (no inter-layer activation) on its experts'
          tokens.  All matmuls keep tokens on the PSUM free dim, so the
          per-expert biases are plain per-partition broadcasts.
  host:   transpose back, scatter rows to token order, out = x + sel.

Compute dtype bf16 (PSUM accumulates fp32); biases/output fp32 paths.
"""

import os
import sys

import numpy as np

for _p in ("/opt/trn_rl_repo", "/root/.axon_site/_ro/trn_rl_repo"):
    if os.path.isdir(_p) and _p not in sys.path:
        sys.path.insert(0, _p)

import ml_dtypes

B, D, S, NB = 4096, 1024, 256, 16
NCORES = 8
EPC = NB // NCORES  # experts per core
KD = D // 128  # d-dim k/m tiles
KS = S // 128  # s-dim tiles

BF16 = ml_dtypes.bfloat16

_programs = {}  # C -> compiled Bacc program
LAST_RESULT = None  # BassKernelResults of the most recent run (for test.py)


N_WARM = 36  # PE warm-up matmuls (HAM p-state ramp) before real data lands


def _build_program(C):
    import concourse.mybir as mybir
    import concourse.tile as tile
    from concourse import bacc

    cdt = mybir.dt.bfloat16
    f32 = mybir.dt.float32
    ident = mybir.ActivationFunctionType.Identity

    nc = bacc.Bacc()
    hT = nc.declare_dram_parameter("hT", [KD, 128, EPC * C], cdt, isOutput=False)
    w1 = nc.declare_dram_parameter("w1", [EPC, KD, 128, S], cdt, isOutput=False)
    w2 = nc.declare_dram_parameter("w2", [EPC, KS, 128, D], cdt, isOutput=False)
    # b_in and b_out packed: bc[e, 0:KS] = b_in tiles, bc[e, KS:KS+KD] = b_out
    bc = nc.declare_dram_parameter("bc", [EPC, KS + KD, 128, 1], f32, isOutput=False)
    yT = nc.declare_dram_parameter("yT", [KD, 128, EPC * C], cdt, isOutput=True)

    HK = KD // 2  # h is loaded in two half-loads of HK d-tiles each

    with tile.TileContext(nc) as tc:
        with (
            tc.tile_pool(name="bias", bufs=1) as bias_pool,
            tc.tile_pool(name="h", bufs=1) as h_pool,
            tc.tile_pool(name="w1p", bufs=1) as w1_pool,
            tc.tile_pool(name="w2p", bufs=1) as w2_pool,
            tc.tile_pool(name="hid", bufs=4) as hid_pool,
            tc.tile_pool(name="yout", bufs=2) as y_pool,
            tc.tile_pool(name="ps1", bufs=2, space="PSUM") as ps1_pool,
            tc.tile_pool(name="ps2", bufs=6, space="PSUM") as ps2_pool,
            tc.tile_pool(name="warm", bufs=1) as warm_pool,
        ):
            # Dummy matmuls keep the PE busy from t=0 so the HAM throttle is
            # fully ramped by the time the first real operands arrive.  The
            # warm PSUM tile borrows a ps2 slot (released before layer 2).
            wz = warm_pool.tile([128, 64], cdt, tag="wz")
            nc.gpsimd.memset(wz[:], 0)
            wps = ps2_pool.tile([128, C], f32, name="wps", tag="ps")
            for _ in range(N_WARM):
                nc.tensor.matmul(
                    wps[0:64, 0:64], lhsT=wz[:, 0:64], rhs=wz[:], start=True, stop=True
                )
            # Biases ride the SWDGE (gpsimd) path: the HWDGE rings are the
            # serial resource, Pool is idle.
            NB_COL = KS + KD
            bct = bias_pool.tile([128, EPC * NB_COL], f32, tag="bc")
            nc.gpsimd.dma_start(
                out=bct[:].rearrange("p (e t) -> p e t", e=EPC),
                in_=bc[:, :, :, 0].rearrange("e t p -> p e t"),
            )

            def b1_ap(e, t):
                return bct[:, e * NB_COL + t : e * NB_COL + t + 1]

            def b2_ap(e, k):
                return bct[:, e * NB_COL + KS + k : e * NB_COL + KS + k + 1]

            # Activations: graduated chunks (1,1,2,4 d-tiles) so the PE can
            # start as soon as the first small chunks land.
            H_CHUNKS = [(0, 2), (2, 4), (4, 6), (6, 8)]
            h_pool_tiles = [
                h_pool.tile(
                    [128, (k1 - k0) * EPC * C], cdt, tag=f"h{i}", name=f"h{i}"
                )
                for i, (k0, k1) in enumerate(H_CHUNKS)
            ]

            def load_h(i):
                k0, k1 = H_CHUNKS[i]
                nc.sync.dma_start(
                    out=h_pool_tiles[i][:].rearrange("p (k n) -> p k n", k=k1 - k0),
                    in_=hT[k0:k1].rearrange("k p n -> p k n"),
                )

            def h_slice(k, e):  # rhs [128, C] for d-tile k, expert e
                for i, (k0, k1) in enumerate(H_CHUNKS):
                    if k0 <= k < k1:
                        return h_pool_tiles[i][
                            :, ((k - k0) * EPC + e) * C : ((k - k0) * EPC + e) * C + C
                        ]
                raise AssertionError(k)

            def make_w1(e):
                return w1_pool.tile([128, KD * S], cdt, tag=f"w1_{e}", name=f"w1_{e}")

            def load_w1_part(e, w1t, k0, k1, eng=None):
                (eng or nc.sync).dma_start(
                    out=w1t[:, k0 * S : k1 * S].rearrange("p (k s) -> p k s", k=k1 - k0),
                    in_=w1[e][k0:k1].rearrange("k p s -> p k s"),
                )

            # w2 tile free layout: (q, t, d_within_quarter) — a d-quarter can
            # be loaded on its own so layer 2's m-groups unblock pairwise.
            DQ = D // 4

            def load_w2(e, w2t, eng=None):
                (eng or nc.sync).dma_start(
                    out=w2t[:].rearrange("p (q t d) -> p q t d", q=4, t=KS),
                    in_=w2[e].rearrange("t p (q d) -> p q t d", q=4),
                )

            def load_w2_quarter(e, w2t, q):
                nc.sync.dma_start(
                    out=w2t[:, q * KS * DQ : (q + 1) * KS * DQ].rearrange(
                        "p (t d) -> p t d", t=KS
                    ),
                    in_=w2[e][:, :, q * DQ : (q + 1) * DQ].rearrange("t p d -> p t d"),
                )

            def w2_slice(w2t, t, m):
                q, dd = divmod(m * 128, DQ)
                base = q * KS * DQ + t * DQ + dd
                return w2t[:, base : base + 128]

            # SP ring, in first-need order, fine-grained at the start so the
            # first matmuls unblock as early as possible.
            w1_tiles = [make_w1(0), make_w1(1)]
            w2_tiles = [
                w2_pool.tile([128, KS * D], cdt, tag=f"w2_{e}", name=f"w2_{e}")
                for e in range(EPC)
            ]
            load_h(0)
            load_w1_part(0, w1_tiles[0], 0, 4)
            load_h(1)
            load_h(2)
            load_w1_part(0, w1_tiles[0], 4, 8)
            load_h(3)

            for e in range(EPC):
                # Layer 1: hid^T[s, c] = sum_d W_in[s, d] * h^T[d, c]
                hids = []
                for t in range(KS):
                    ps = ps1_pool.tile([128, C], f32)
                    for k in range(KD):
                        nc.tensor.matmul(
                            ps[:],
                            lhsT=w1_tiles[e][:, k * S + t * 128 : k * S + t * 128 + 128],
                            rhs=h_slice(k, e),
                            start=(k == 0),
                            stop=(k == KD - 1),
                        )
                    if e == 0 and t == 0:
                        load_w2(0, w2_tiles[0])
                    elif e == 0 and t == 1:
                        load_w1_part(1, w1_tiles[1], 0, 4)
                        load_w1_part(1, w1_tiles[1], 4, 8)
                    hid = hid_pool.tile([128, C], cdt)
                    nc.scalar.activation(hid[:], ps[:], ident, bias=b1_ap(e, t))
                    hids.append(hid)

                # Layer 2: y^T[d, c] = sum_s W_out[d, s] * hid^T[s, c]
                # Evictions alternate ACT / DVE into one [128, KD*C] tile;
                # stored in two strided half-DMAs so the tail store is short.
                y_big = y_pool.tile([128, KD * C], cdt)
                for m in range(KD):
                    ps = ps2_pool.tile([128, C], f32)
                    for t in range(KS):
                        nc.tensor.matmul(
                            ps[:],
                            lhsT=w2_slice(w2_tiles[e], t, m),
                            rhs=hids[t][:],
                            start=(t == 0),
                            stop=(t == KS - 1),
                        )
                    if e == 0 and m == 0:
                        load_w2_quarter(1, w2_tiles[1], 0)
                        load_w2_quarter(1, w2_tiles[1], 1)
                    elif e == 0 and m == 2:
                        load_w2_quarter(1, w2_tiles[1], 2)
                        load_w2_quarter(1, w2_tiles[1], 3)
                    dst = y_big[:, m * C : (m + 1) * C]
                    bias_ap = b2_ap(e, m)
                    if m % 2 == 0:
                        nc.scalar.activation(dst, ps[:], ident, bias=bias_ap)
                    else:
                        nc.vector.tensor_scalar_add(dst, ps[:], bias_ap)
                    # Stores alternate between the two idle DMA issuers —
                    # gpsimd (SWDGE) and SP (HWDGE, free once loads are done) —
                    # so tail stores don't serialize on one generator.
                    store_after = {3: (0, 4), 7: (4, 8)} if e == 0 else {
                        1: (0, 2), 3: (2, 4), 5: (4, 6), 7: (6, 8)
                    }
                    if m in store_after:
                        k0, k1 = store_after[m]
                        issuer = nc.gpsimd if (m // 2) % 2 == 0 else nc.sync
                        issuer.dma_start(
                            out=yT[k0:k1, :, e * C : (e + 1) * C]
                            .rearrange("k p n -> p k n"),
                            in_=y_big[:, k0 * C : k1 * C]
                            .rearrange("p (k n) -> p k n", k=k1 - k0),
                        )

    nc.compile()
    return nc


def kernel(x, y_index, W_in, b_in, W_out, b_out):
    global LAST_RESULT
    from concourse.bass_utils import run_bass_kernel_spmd

    x = np.asarray(x, dtype=np.float32)
    W_in = np.asarray(W_in, dtype=np.float32)
    b_in = np.asarray(b_in, dtype=np.float32)
    W_out = np.asarray(W_out, dtype=np.float32)
    b_out = np.asarray(b_out, dtype=np.float32)
    eidx = np.asarray(y_index).reshape(-1).astype(np.int64)

    counts = np.bincount(eidx, minlength=NB)
    C = max(276, int(-(-counts.max() // 4) * 4))  # capacity per expert

    if C > 512:
        # Extreme expert skew would overflow a PSUM bank (512 f32 free dim);
        # fall back to exact host math rather than ship a broken program.
        out = np.empty_like(x)
        h_full = np.maximum(x, 0.0)
        for e in range(NB):
            m = eidx == e
            if m.any():
                hid = h_full[m] @ W_in[e].T + b_in[e]
                out[m] = x[m] + hid @ W_out[e].T + b_out[e]
        return out

    # --- host dispatch: group tokens by expert ---------------------------
    order = np.argsort(eidx, kind="stable")
    starts = np.zeros(NB + 1, dtype=np.int64)
    np.cumsum(counts, out=starts[1:])

    h = np.maximum(x, 0.0)
    Xg = np.zeros((NB, C, D), dtype=np.float32)
    for e in range(NB):
        toks = order[starts[e] : starts[e + 1]]
        Xg[e, : counts[e]] = h[toks]

    # [NB, C, D] -> per core [D, EPC*C] -> [KD, 128, EPC*C]
    hT_all = (
        Xg.reshape(NCORES, EPC * C, D)
        .transpose(0, 2, 1)
        .reshape(NCORES, KD, 128, EPC * C)
        .astype(BF16)
    )
    w1_all = (
        W_in.transpose(0, 2, 1).reshape(NCORES, EPC, KD, 128, S).astype(BF16)
    )
    w2_all = (
        W_out.transpose(0, 2, 1).reshape(NCORES, EPC, KS, 128, D).astype(BF16)
    )
    bc_all = np.concatenate(
        [b_in.reshape(NB, KS, 128, 1), b_out.reshape(NB, KD, 128, 1)], axis=1
    ).reshape(NCORES, EPC, KS + KD, 128, 1)

    if C not in _programs:
        _programs[C] = _build_program(C)
    nc = _programs[C]

    in_maps = [
        {
            "hT": np.ascontiguousarray(hT_all[i]),
            "w1": np.ascontiguousarray(w1_all[i]),
            "w2": np.ascontiguousarray(w2_all[i]),
            "bc": np.ascontiguousarray(bc_all[i]),
        }
        for i in range(NCORES)
    ]

    trace = bool(int(os.environ.get("KERNEL_TRACE", "0")))
    res = run_bass_kernel_spmd(nc, in_maps, list(range(NCORES)), trace=trace)
    LAST_RESULT = res

    # --- host gather: transpose back, scatter to token order -------------
    out = np.empty_like(x)
    Yg = np.stack(
        [
            r["yT"].reshape(D, EPC * C).astype(np.float32)
            for r in res.results
        ]
    )  # [NCORES, D, EPC*C]
    Yg = Yg.transpose(0, 2, 1).reshape(NB, C, D)
    for e in range(NB):
        toks = order[starts[e] : starts[e + 1]]
        out[toks] = x[toks] + Yg[e, : counts[e]]
    return out



# revision 2
# speedup vs baseline: 1.0680x; 1.0680x over previous
"""MoE routing kernel (nn_DecFCSwitch) for 8 Trainium2 NeuronCores.

Reference computes all 16 expert branches for every token and then
selects one per token.  Only the selected branch matters, so:

  host:   sort tokens by expert, pad each expert's tokens to capacity C,
          relu(x) (the residual add also stays on host), transpose so
          the feature dim lands on SBUF partitions, quantize to fp8e3
          (e3m4) with power-of-two scales.
  device: expert-parallel SPMD — core i owns experts {2i, 2i+1} and runs
          a 2-layer MLP (no inter-layer activation) on its experts'
          tokens.  Matmuls run in fp8e3 (PSUM accumulates fp32); the
          layer-1 eviction requantizes the hidden to fp8e3 on the ACT
          engine (scale folded into the activation affine), layer-2
          evictions add the (pre-scaled) output bias and write bf16.
  host:   transpose back, un-scale, scatter rows to token order,
          out = x + sel.

Scales (all powers of two, exact): h*2, W_in*256, hid*4, W_out*128.
Layer-1 ACT: hid8 = fp8(psum/128 + 4*b_in).  Layer-2 evict: y = psum +
512*b_out (bf16); host divides by 512.  Predicted rel err ~6e-3 vs the
2e-2 gate (fp8 quantization; the residual x dominates the output norm).
"""

import os
import sys

import numpy as np

for _p in ("/opt/trn_rl_repo", "/root/.axon_site/_ro/trn_rl_repo"):
    if os.path.isdir(_p) and _p not in sys.path:
        sys.path.insert(0, _p)

import ml_dtypes

B, D, S, NB = 4096, 1024, 256, 16
NCORES = 8
EPC = NB // NCORES  # experts per core
KD = D // 128  # d-dim k/m tiles
KS = S // 128  # s-dim tiles
DQ = D // 4  # d-quarter (w2 block width in elements)

E3 = ml_dtypes.float8_e3m4
BF16 = ml_dtypes.bfloat16

SH = 2.0  # h scale
SW1 = 256.0  # W_in scale
SH2 = 4.0  # hidden scale
SW2 = 128.0  # W_out scale
# layer-1 ACT: out = psum * (SH2/(SH*SW1)) + SH2*b_in
S1 = SH2 / (SH * SW1)
# layer-2 evict: y = psum + (SH2*SW2)*b_out ; host divides by SH2*SW2
SY = SH2 * SW2

_programs = {}  # C -> compiled Bacc program
LAST_RESULT = None  # BassKernelResults of the most recent run (for test.py)


N_WARM = 44  # PE warm-up matmuls (HAM p-state ramp) before real data lands


def _build_program(C):
    import concourse.mybir as mybir
    import concourse.tile as tile
    from concourse import bacc

    f8 = mybir.dt.float8e3
    bf = mybir.dt.bfloat16
    f32 = mybir.dt.float32
    ident = mybir.ActivationFunctionType.Identity

    nc = bacc.Bacc()
    hp = nc.declare_dram_parameter("hp", [KD, 128, EPC * C], f8, isOutput=False)
    w1 = nc.declare_dram_parameter("w1", [EPC, 128, KD * S], f8, isOutput=False)
    w2 = nc.declare_dram_parameter("w2", [EPC, 128, 4 * KS * DQ], f8, isOutput=False)
    # bc[e, 0:KS] = SH2*b_in tiles, bc[e, KS:KS+KD] = SY*b_out tiles
    bc = nc.declare_dram_parameter("bc", [EPC, KS + KD, 128, 1], f32, isOutput=False)
    yT = nc.declare_dram_parameter("yT", [KD, 128, EPC * C], bf, isOutput=True)

    NB_COL = KS + KD

    with tile.TileContext(nc) as tc:
        with (
            tc.tile_pool(name="bias", bufs=1) as bias_pool,
            tc.tile_pool(name="h", bufs=1) as h_pool,
            tc.tile_pool(name="w1p", bufs=1) as w1_pool,
            tc.tile_pool(name="w2p", bufs=1) as w2_pool,
            tc.tile_pool(name="hid", bufs=4) as hid_pool,
            tc.tile_pool(name="yout", bufs=2) as y_pool,
            tc.tile_pool(name="ps1", bufs=2, space="PSUM") as ps1_pool,
            tc.tile_pool(name="ps2", bufs=6, space="PSUM") as ps2_pool,
            tc.tile_pool(name="warm", bufs=1) as warm_pool,
        ):
            # Dummy matmuls keep the PE busy from t=0 so the HAM throttle is
            # fully ramped by the time the first real operands arrive.
            wz = warm_pool.tile([128, 64], f8, tag="wz")
            nc.gpsimd.memset(wz[:], 0)
            wps = ps2_pool.tile([128, C], f32, name="wps", tag="ps")
            for _ in range(N_WARM):
                nc.tensor.matmul(
                    wps[0:64, 0:64], lhsT=wz[:, 0:64], rhs=wz[:], start=True, stop=True
                )
            # Biases ride the SWDGE (gpsimd) path: HWDGE is the serial
            # resource for the big loads, Pool is idle.
            bct = bias_pool.tile([128, EPC * NB_COL], f32, tag="bc")
            nc.gpsimd.dma_start(
                out=bct[:].rearrange("p (e t) -> p e t", e=EPC),
                in_=bc[:, :, :, 0].rearrange("e t p -> p e t"),
            )

            def b1_ap(e, t):
                return bct[:, e * NB_COL + t : e * NB_COL + t + 1]

            def b2_ap(e, m):
                return bct[:, e * NB_COL + KS + m : e * NB_COL + KS + m + 1]

            # Activations: graduated chunks (2,2,2,2 d-tiles) so the PE can
            # start as soon as the first chunks land.
            H_CHUNKS = [(0, 2), (2, 4), (4, 6), (6, 8)]
            h_tiles = [
                h_pool.tile([128, (k1 - k0) * EPC * C], f8, tag=f"h{i}", name=f"h{i}")
                for i, (k0, k1) in enumerate(H_CHUNKS)
            ]

            def load_h(i, eng=None):
                k0, k1 = H_CHUNKS[i]
                (eng or nc.sync).dma_start(
                    out=h_tiles[i][:].rearrange("p (k n) -> p k n", k=k1 - k0),
                    in_=hp[k0:k1].rearrange("k p n -> p k n"),
                )

            def h_slice(k, e):  # rhs [128, C] for d-tile k, expert e
                for i, (k0, k1) in enumerate(H_CHUNKS):
                    if k0 <= k < k1:
                        return h_tiles[i][
                            :, ((k - k0) * EPC + e) * C : ((k - k0) * EPC + e) * C + C
                        ]
                raise AssertionError(k)

            w1_tiles = [
                w1_pool.tile([128, KD * S], f8, tag=f"w1_{e}", name=f"w1_{e}")
                for e in range(EPC)
            ]
            w2_tiles = [
                w2_pool.tile([128, 4 * KS * DQ], f8, tag=f"w2_{e}", name=f"w2_{e}")
                for e in range(EPC)
            ]

            def load_w1_part(e, k0, k1, eng=None):
                # free run per partition: (k1-k0)*S fp8 bytes, >= 512B for 2 tiles
                (eng or nc.sync).dma_start(
                    out=w1_tiles[e][:, k0 * S : k1 * S],
                    in_=w1[e][:, k0 * S : k1 * S],
                )

            def load_w2_half(e, q0, eng=None):
                # quarters q0, q0+1: contiguous [128, 1024] block, 1KB runs
                (eng or nc.sync).dma_start(
                    out=w2_tiles[e][:, q0 * 2 * KS * 128 : (q0 + 2) * 2 * KS * 128],
                    in_=w2[e][:, q0 * 2 * KS * 128 : (q0 + 2) * 2 * KS * 128],
                )

            def w2_slice(e, t, m):
                q, r = divmod(m * 128, DQ)
                base = q * (KS * DQ) + t * DQ + r
                return w2_tiles[e][:, base : base + 128]

            # Loads in first-need order.  h0/w1(e0) first so the PE unblocks
            # ASAP; expert-1 weights ride the SWDGE path to keep HWDGE free
            # for the stores that interleave with them later.
            load_h(0)
            load_w1_part(0, 0, 4)
            load_h(1)
            load_w1_part(0, 4, 8)
            load_h(2)
            load_h(3)
            load_w2_half(0, 0)
            load_w2_half(0, 2)
            load_w1_part(1, 0, 8, eng=nc.gpsimd)
            load_w2_half(1, 0, eng=nc.gpsimd)
            load_w2_half(1, 2, eng=nc.gpsimd)

            for e in range(EPC):
                # Layer 1: hid^T[s, c] = sum_d W_in[s, d] * h^T[d, c]
                hids = []
                for t in range(KS):
                    ps = ps1_pool.tile([128, C], f32)
                    for k in range(KD):
                        nc.tensor.matmul(
                            ps[:],
                            lhsT=w1_tiles[e][:, k * S + t * 128 : k * S + t * 128 + 128],
                            rhs=h_slice(k, e),
                            start=(k == 0),
                            stop=(k == KD - 1),
                        )
                    hid = hid_pool.tile([128, C], f8)
                    nc.scalar.activation(hid[:], ps[:], ident, bias=b1_ap(e, t), scale=S1)
                    hids.append(hid)

                # Layer 2: y^T[d, c] = sum_s W_out[d, s] * hid^T[s, c]
                y_big = y_pool.tile([128, KD * C], bf)
                for m in range(KD):
                    ps = ps2_pool.tile([128, C], f32)
                    for t in range(KS):
                        nc.tensor.matmul(
                            ps[:],
                            lhsT=w2_slice(e, t, m),
                            rhs=hids[t][:],
                            start=(t == 0),
                            stop=(t == KS - 1),
                        )
                    dst = y_big[:, m * C : (m + 1) * C]
                    bias_ap = b2_ap(e, m)
                    # Evictions alternate ACT / DVE so neither engine gates
                    # the PE; both are plain adds (scales pre-folded on host).
                    if m % 2 == 0:
                        nc.scalar.activation(dst, ps[:], ident, bias=bias_ap)
                    else:
                        nc.vector.tensor_scalar_add(dst, ps[:], bias_ap)
                    # Staged stores; issuers alternate between the HWDGE
                    # (sync) and SWDGE (gpsimd) generators.
                    store_after = (
                        {3: (0, 4), 7: (4, 8)}
                        if e == 0
                        else {3: (0, 4), 5: (4, 6), 7: (6, 8)}
                    )
                    if m in store_after:
                        k0, k1 = store_after[m]
                        issuer = nc.gpsimd if (e == 0 and m == 3) else nc.sync
                        issuer.dma_start(
                            out=yT[k0:k1, :, e * C : (e + 1) * C].rearrange(
                                "k p n -> p k n"
                            ),
                            in_=y_big[:, k0 * C : k1 * C].rearrange(
                                "p (k n) -> p k n", k=k1 - k0
                            ),
                        )

    nc.compile()
    return nc


def kernel(x, y_index, W_in, b_in, W_out, b_out):
    global LAST_RESULT
    from concourse.bass_utils import run_bass_kernel_spmd

    x = np.asarray(x, dtype=np.float32)
    W_in = np.asarray(W_in, dtype=np.float32)
    b_in = np.asarray(b_in, dtype=np.float32)
    W_out = np.asarray(W_out, dtype=np.float32)
    b_out = np.asarray(b_out, dtype=np.float32)
    eidx = np.asarray(y_index).reshape(-1).astype(np.int64)

    counts = np.bincount(eidx, minlength=NB)
    C = max(276, int(-(-counts.max() // 4) * 4))  # capacity per expert

    if C > 512:
        # Extreme expert skew would overflow a PSUM bank (512 f32 free dim);
        # fall back to exact host math rather than ship a broken program.
        out = np.empty_like(x)
        h_full = np.maximum(x, 0.0)
        for e in range(NB):
            m = eidx == e
            if m.any():
                hid = h_full[m] @ W_in[e].T + b_in[e]
                out[m] = x[m] + hid @ W_out[e].T + b_out[e]
        return out

    # --- host dispatch: group tokens by expert ---------------------------
    order = np.argsort(eidx, kind="stable")
    starts = np.zeros(NB + 1, dtype=np.int64)
    np.cumsum(counts, out=starts[1:])

    h = np.maximum(x, 0.0)
    Xg = np.zeros((NB, C, D), dtype=np.float32)
    for e in range(NB):
        toks = order[starts[e] : starts[e + 1]]
        Xg[e, : counts[e]] = h[toks]

    # [NB, C, D] -> per core [KD, 128, EPC*C], fp8e3 with h*SH scaling
    hp_all = (
        (Xg * SH)
        .astype(E3)
        .reshape(NCORES, EPC * C, KD, 128)
        .transpose(0, 2, 3, 1)
    )
    # W_in [NB, S, D] -> [NCORES, EPC, 128(dp), KD*S], value W_in[s, k*128+p]*SW1
    w1_all = (
        (W_in * SW1)
        .astype(E3)
        .reshape(NCORES, EPC, S, KD, 128)
        .transpose(0, 1, 4, 3, 2)
        .reshape(NCORES, EPC, 128, KD * S)
    )
    # W_out [NB, D, S] -> [NCORES, EPC, 128(sp), (q, t, dq)], W_out[q*DQ+dq, t*128+p]*SW2
    w2_all = (
        (W_out * SW2)
        .astype(E3)
        .reshape(NCORES, EPC, 4, DQ, KS, 128)
        .transpose(0, 1, 5, 2, 4, 3)
        .reshape(NCORES, EPC, 128, 4 * KS * DQ)
    )
    bc_all = np.concatenate(
        [
            (SH2 * b_in).reshape(NB, KS, 128, 1),
            (SY * b_out).reshape(NB, KD, 128, 1),
        ],
        axis=1,
    ).reshape(NCORES, EPC, KS + KD, 128, 1)

    if C not in _programs:
        _programs[C] = _build_program(C)
    nc = _programs[C]

    in_maps = [
        {
            "hp": np.ascontiguousarray(hp_all[i]),
            "w1": np.ascontiguousarray(w1_all[i]),
            "w2": np.ascontiguousarray(w2_all[i]),
            "bc": np.ascontiguousarray(bc_all[i]),
        }
        for i in range(NCORES)
    ]

    trace = bool(int(os.environ.get("KERNEL_TRACE", "0")))
    res = run_bass_kernel_spmd(nc, in_maps, list(range(NCORES)), trace=trace)
    LAST_RESULT = res

    # --- host gather: transpose back, un-scale, scatter to token order ---
    out = np.empty_like(x)
    Yg = np.stack(
        [r["yT"].reshape(D, EPC * C).astype(np.float32) for r in res.results]
    )  # [NCORES, D, EPC*C]
    Yg = Yg.transpose(0, 2, 1).reshape(NB, C, D) * (1.0 / SY)
    for e in range(NB):
        toks = order[starts[e] : starts[e + 1]]
        out[toks] = x[toks] + Yg[e, : counts[e]]
    return out


# revision 6
# speedup vs baseline: 1.1805x; 1.1053x over previous
"""MoE routing kernel (nn_DecFCSwitch) for 8 Trainium2 NeuronCores.

Reference computes all 16 expert branches for every token and then
selects one per token.  Only the selected branch matters, so:

  host:   sort tokens by expert, pad each expert's tokens to capacity C,
          relu(x) (the residual add also stays on host), transpose so
          the feature dim lands on SBUF partitions, quantize to fp8e3
          (e3m4) with power-of-two scales.
  device: expert-parallel SPMD — core i owns experts {2i, 2i+1} and runs
          a 2-layer MLP (no inter-layer activation) on its experts'
          tokens.  Matmuls run in fp8e3 (PSUM accumulates fp32); the
          layer-1 eviction requantizes the hidden to fp8e3 on the ACT
          engine (scale folded into the activation affine), layer-2
          evictions add the (pre-scaled) output bias and write bf16.
  host:   transpose back, un-scale, scatter rows to token order,
          out = x + sel.

Scales (all powers of two, exact): h*2, W_in*256, hid*4, W_out*128.
Layer-1 ACT: hid8 = fp8(psum/128 + 4*b_in).  Layer-2 evict: y = psum +
512*b_out (bf16); host divides by 512.  Predicted rel err ~6e-3 vs the
2e-2 gate (fp8 quantization; the residual x dominates the output norm).
"""

import os
import sys

import numpy as np

for _p in ("/opt/trn_rl_repo", "/root/.axon_site/_ro/trn_rl_repo"):
    if os.path.isdir(_p) and _p not in sys.path:
        sys.path.insert(0, _p)

import ml_dtypes

B, D, S, NB = 4096, 1024, 256, 16
NCORES = 8
EPC = NB // NCORES  # experts per core
KD = D // 128  # d-dim k/m tiles
KS = S // 128  # s-dim tiles
DQ = D // 4  # d-quarter (w2 block width in elements)

E3 = ml_dtypes.float8_e3m4
BF16 = ml_dtypes.bfloat16

SH = 2.0  # h scale
SW1 = 256.0  # W_in scale
SH2 = 4.0  # hidden scale
SW2 = 128.0  # W_out scale
# layer-1 ACT: out = psum * (SH2/(SH*SW1)) + SH2*b_in
S1 = SH2 / (SH * SW1)
# layer-2 evict: y = psum + (SH2*SW2)*b_out ; host divides by SH2*SW2
SY = SH2 * SW2

_programs = {}  # C -> compiled Bacc program
LAST_RESULT = None  # BassKernelResults of the most recent run (for test.py)


N_WARM = 44  # PE warm-up matmuls (HAM p-state ramp) before real data lands


def _build_program(C):
    import concourse.mybir as mybir
    import concourse.tile as tile
    from concourse import bacc

    f8 = mybir.dt.float8e3
    bf = mybir.dt.bfloat16
    f32 = mybir.dt.float32
    ident = mybir.ActivationFunctionType.Identity

    nc = bacc.Bacc()
    hp = nc.declare_dram_parameter("hp", [KD, 128, EPC * C], f8, isOutput=False)
    w1 = nc.declare_dram_parameter("w1", [EPC, 128, KD * S], f8, isOutput=False)
    w2 = nc.declare_dram_parameter("w2", [EPC, 128, 4 * KS * DQ], f8, isOutput=False)
    # bc[:, e*(KS+KD) + t]: t<KS -> SH2*b_in tile t; else SY*b_out tile t-KS
    bc = nc.declare_dram_parameter("bc", [128, EPC * (KS + KD)], f32, isOutput=False)
    yT = nc.declare_dram_parameter("yT", [KD, 128, EPC * C], bf, isOutput=True)

    NB_COL = KS + KD

    with tile.TileContext(nc) as tc:
        with (
            tc.tile_pool(name="bias", bufs=1) as bias_pool,
            tc.tile_pool(name="h", bufs=1) as h_pool,
            tc.tile_pool(name="w1p", bufs=1) as w1_pool,
            tc.tile_pool(name="w2p", bufs=1) as w2_pool,
            tc.tile_pool(name="hid", bufs=4) as hid_pool,
            tc.tile_pool(name="yout", bufs=2) as y_pool,
            tc.tile_pool(name="ps1", bufs=4, space="PSUM") as ps1_pool,
            tc.tile_pool(name="ps2", bufs=4, space="PSUM") as ps2_pool,
            tc.tile_pool(name="warm", bufs=1) as warm_pool,
        ):
            # Dummy matmuls keep the PE busy from t=0 so the HAM throttle is
            # fully ramped by the time the first real operands arrive.
            wz = warm_pool.tile([128, 64], f8, tag="wz")
            nc.gpsimd.memset(wz[:], 0)
            wact = warm_pool.tile([128, 1], bf, tag="wact")
            # Dummy activation fires the ACT table load (1283 ns) during the
            # DMA lead-in instead of on the first hid eviction.
            nc.scalar.activation(wact[:], wz[:, 0:1], ident)
            wps = ps2_pool.tile([128, C], f32, name="wps", tag="ps")
            for _ in range(N_WARM):
                nc.tensor.matmul(
                    wps[0:64, 0:64], lhsT=wz[:, 0:64], rhs=wz[:], start=True, stop=True
                )
            # Bias vector is tiny and packed contiguously; SWDGE (gpsimd)
            # path keeps it off the serial HWDGE resource.
            bct = bias_pool.tile([128, EPC * NB_COL], f32, tag="bc")
            nc.gpsimd.dma_start(out=bct[:], in_=bc[:, :])

            def b1_ap(e, t):
                return bct[:, e * NB_COL + t : e * NB_COL + t + 1]

            def b2_ap(e, m):
                return bct[:, e * NB_COL + KS + m : e * NB_COL + KS + m + 1]

            # Activations in chunks of 2 d-tiles so the PE can start as soon
            # as the first chunks land.
            H_CHUNKS = [(0, 2), (2, 4), (4, 6), (6, 8)]
            h_tiles = [
                h_pool.tile([128, (k1 - k0) * EPC * C], f8, tag=f"h{i}", name=f"h{i}")
                for i, (k0, k1) in enumerate(H_CHUNKS)
            ]

            def load_h(i, eng=None):
                k0, k1 = H_CHUNKS[i]
                (eng or nc.sync).dma_start(
                    out=h_tiles[i][:].rearrange("p (k n) -> p k n", k=k1 - k0),
                    in_=hp[k0:k1].rearrange("k p n -> p k n"),
                )

            def h_slice(k, e):  # rhs [128, C] for d-tile k, expert e
                for i, (k0, k1) in enumerate(H_CHUNKS):
                    if k0 <= k < k1:
                        return h_tiles[i][
                            :, ((k - k0) * EPC + e) * C : ((k - k0) * EPC + e) * C + C
                        ]
                raise AssertionError(k)

            w1_tiles = [
                w1_pool.tile([128, KD * S], f8, tag=f"w1_{e}", name=f"w1_{e}")
                for e in range(EPC)
            ]
            w2_tiles = [
                w2_pool.tile([128, 4 * KS * DQ], f8, tag=f"w2_{e}", name=f"w2_{e}")
                for e in range(EPC)
            ]

            def load_w1_part(e, k0, k1, eng=None):
                # free run per partition: (k1-k0)*S fp8 bytes, >= 512B for 2 tiles
                (eng or nc.sync).dma_start(
                    out=w1_tiles[e][:, k0 * S : k1 * S],
                    in_=w1[e][:, k0 * S : k1 * S],
                )

            def load_w2_half(e, q0, eng=None):
                # quarters q0, q0+1: contiguous [128, 1024] block, 1KB runs
                (eng or nc.sync).dma_start(
                    out=w2_tiles[e][:, q0 * 2 * KS * 128 : (q0 + 2) * 2 * KS * 128],
                    in_=w2[e][:, q0 * 2 * KS * 128 : (q0 + 2) * 2 * KS * 128],
                )

            def w2_slice(e, t, m):
                q, r = divmod(m * 128, DQ)
                base = q * (KS * DQ) + t * DQ + r
                return w2_tiles[e][:, base : base + 128]

            # Loads in first-need order.  PE consumes h/w1 chunk-by-chunk
            # (L1 e0 then e1), then w2(e0), w2(e1).  Expert-1 weights ride
            # the SWDGE (gpsimd) path — its desc-gen is serial on Pool but
            # the HWDGE ring is the scarcer resource.
            load_h(0)
            load_w1_part(0, 0, 4)
            load_h(1)
            load_w1_part(0, 4, 8)
            load_h(2)
            load_h(3)
            load_w2_half(0, 0)
            load_w2_half(0, 2)
            load_w1_part(1, 0, 8, eng=nc.gpsimd)
            load_w2_half(1, 0, eng=nc.gpsimd)
            load_w2_half(1, 2, eng=nc.gpsimd)

            # ---- Layer 1 for both experts (k-interleaved so each arriving
            # h chunk feeds 4 matmuls immediately), then Layer 2 ----------
            all_hids = []
            for e in range(EPC):
                pss = [
                    ps1_pool.tile([128, C], f32, name=f"ps1_{e}_{t}", tag="ps1")
                    for t in range(KS)
                ]
                for k in range(KD):
                    for t in range(KS):
                        nc.tensor.matmul(
                            pss[t][:],
                            lhsT=w1_tiles[e][:, k * S + t * 128 : k * S + t * 128 + 128],
                            rhs=h_slice(k, e),
                            start=(k == 0),
                            stop=(k == KD - 1),
                        )
                hids = []
                for t in range(KS):
                    hid = hid_pool.tile([128, C], f8)
                    nc.scalar.activation(
                        hid[:], pss[t][:], ident, bias=b1_ap(e, t), scale=S1
                    )
                    hids.append(hid)
                all_hids.append(hids)

            for e in range(EPC):
                hids = all_hids[e]
                # Layer 2: y^T[d, c] = sum_s W_out[d, s] * hid^T[s, c]
                y_big = y_pool.tile([128, KD * C], bf)
                for m in range(KD):
                    ps = ps2_pool.tile([128, C], f32)
                    for t in range(KS):
                        nc.tensor.matmul(
                            ps[:],
                            lhsT=w2_slice(e, t, m),
                            rhs=hids[t][:],
                            start=(t == 0),
                            stop=(t == KS - 1),
                        )
                    dst = y_big[:, m * C : (m + 1) * C]
                    bias_ap = b2_ap(e, m)
                    # Evictions alternate ACT / DVE so neither engine gates
                    # the PE; both are plain adds (scales pre-folded on host).
                    if m % 2 == 0:
                        nc.scalar.activation(dst, ps[:], ident, bias=bias_ap)
                    else:
                        nc.vector.tensor_scalar_add(dst, ps[:], bias_ap)
                    # Staged stores; issuers alternate between the HWDGE
                    # (sync) and SWDGE (gpsimd) generators; the final store
                    # is the smallest so the tail transfer is short.
                    store_after = (
                        {3: (0, 4), 7: (4, 8)}
                        if e == 0
                        else {3: (0, 4), 5: (4, 6), 7: (6, 8)}
                    )
                    if m in store_after:
                        k0, k1 = store_after[m]
                        issuer = nc.gpsimd if (e == 0 and m == 3) else nc.sync
                        issuer.dma_start(
                            out=yT[k0:k1, :, e * C : (e + 1) * C].rearrange(
                                "k p n -> p k n"
                            ),
                            in_=y_big[:, k0 * C : k1 * C].rearrange(
                                "p (k n) -> p k n", k=k1 - k0
                            ),
                        )

    nc.compile()
    return nc


def kernel(x, y_index, W_in, b_in, W_out, b_out):
    global LAST_RESULT
    from concourse.bass_utils import run_bass_kernel_spmd

    x = np.asarray(x, dtype=np.float32)
    W_in = np.asarray(W_in, dtype=np.float32)
    b_in = np.asarray(b_in, dtype=np.float32)
    W_out = np.asarray(W_out, dtype=np.float32)
    b_out = np.asarray(b_out, dtype=np.float32)
    eidx = np.asarray(y_index).reshape(-1).astype(np.int64)

    counts = np.bincount(eidx, minlength=NB)
    C = max(276, int(-(-counts.max() // 4) * 4))  # capacity per expert

    if C > 512:
        # Extreme expert skew would overflow a PSUM bank (512 f32 free dim);
        # fall back to exact host math rather than ship a broken program.
        out = np.empty_like(x)
        h_full = np.maximum(x, 0.0)
        for e in range(NB):
            m = eidx == e
            if m.any():
                hid = h_full[m] @ W_in[e].T + b_in[e]
                out[m] = x[m] + hid @ W_out[e].T + b_out[e]
        return out

    # --- host dispatch: group tokens by expert ---------------------------
    order = np.argsort(eidx, kind="stable")
    starts = np.zeros(NB + 1, dtype=np.int64)
    np.cumsum(counts, out=starts[1:])

    h = np.maximum(x, 0.0)
    Xg = np.zeros((NB, C, D), dtype=np.float32)
    for e in range(NB):
        toks = order[starts[e] : starts[e + 1]]
        Xg[e, : counts[e]] = h[toks]

    # [NB, C, D] -> per core [KD, 128, EPC*C], fp8e3 with h*SH scaling
    hp_all = (
        (Xg * SH)
        .astype(E3)
        .reshape(NCORES, EPC * C, KD, 128)
        .transpose(0, 2, 3, 1)
    )
    # W_in [NB, S, D] -> [NCORES, EPC, 128(dp), KD*S], value W_in[s, k*128+p]*SW1
    w1_all = (
        (W_in * SW1)
        .astype(E3)
        .reshape(NCORES, EPC, S, KD, 128)
        .transpose(0, 1, 4, 3, 2)
        .reshape(NCORES, EPC, 128, KD * S)
    )
    # W_out [NB, D, S] -> [NCORES, EPC, 128(sp), (q, t, dq)], W_out[q*DQ+dq, t*128+p]*SW2
    w2_all = (
        (W_out * SW2)
        .astype(E3)
        .reshape(NCORES, EPC, 4, DQ, KS, 128)
        .transpose(0, 1, 5, 2, 4, 3)
        .reshape(NCORES, EPC, 128, 4 * KS * DQ)
    )
    # [NCORES, 128, EPC*(KS+KD)] contiguous per partition
    bc_all = (
        np.concatenate(
            [
                (SH2 * b_in).reshape(NB, KS, 128),
                (SY * b_out).reshape(NB, KD, 128),
            ],
            axis=1,
        )
        .reshape(NCORES, EPC * (KS + KD), 128)
        .transpose(0, 2, 1)
    )

    if C not in _programs:
        _programs[C] = _build_program(C)
    nc = _programs[C]

    in_maps = [
        {
            "hp": np.ascontiguousarray(hp_all[i]),
            "w1": np.ascontiguousarray(w1_all[i]),
            "w2": np.ascontiguousarray(w2_all[i]),
            "bc": np.ascontiguousarray(bc_all[i]),
        }
        for i in range(NCORES)
    ]

    trace = bool(int(os.environ.get("KERNEL_TRACE", "0")))
    res = run_bass_kernel_spmd(nc, in_maps, list(range(NCORES)), trace=trace)
    LAST_RESULT = res

    # --- host gather: transpose back, un-scale, scatter to token order ---
    out = np.empty_like(x)
    Yg = np.stack(
        [r["yT"].reshape(D, EPC * C).astype(np.float32) for r in res.results]
    )  # [NCORES, D, EPC*C]
    Yg = Yg.transpose(0, 2, 1).reshape(NB, C, D) * (1.0 / SY)
    for e in range(NB):
        toks = order[starts[e] : starts[e + 1]]
        out[toks] = x[toks] + Yg[e, : counts[e]]
    return out
